# revision 1
# baseline (speedup 1.0000x reference)
"""Single-head cross-attention on 8 NeuronCores, data-parallel over batch.

Math per core (batch element b):
    q = x @ Wq + bq;  k = enc @ Wk + bk;  v = enc @ Wv + bv
    out = softmax(q k^T / sqrt(H)) @ v @ Wp + bp

Layout strategy (no on-chip transposes anywhere):
    host:    xT[E,T], encT[E,S] (pre-transposed), Wq' = Wq/sqrt(H)
    qT[h,t] = Wq'^T-tiles as lhsT, xT as rhs          (+bq' per-partition)
    kT[h,s] = Wk-tiles  as lhsT, encT as rhs          (+bk  per-partition)
    v[s,h]  = encT-tiles as lhsT, Wv as rhs
    ST[s,t] = kT-tiles  as lhsT, qT as rhs            (scores, transposed)
    Ex      = exp(ST)          (no max subtraction; scores are O(1) here,
                                softmax is shift-invariant so result matches)
    r[t]    = ones^T column matmuls over Ex s-tiles   ([t,1] per t-tile)
    OT[h,t] = v-tiles   as lhsT, Ex as rhs            (unnormalized)
    Y[t,e]  = OT-tiles  as lhsT, Wp as rhs, scaled by 1/r[t] on copy-out
    bv/bp are folded into a host-side rank-1 add: softmax rows sum to 1, so
    P@(v + 1 bv^T) @ Wp + bp = P@v@Wp + 1 (bv@Wp + bp)^T exactly.

All matmuls run in float32r (fp32 bits, fast PE mode, 1 cycle/row at
N>=256; measured faster than bf16 on hw: 389ns vs 454ns per 512-col
matmul, and the BIR verifier forbids mixing fp32r with bf16 operands).
The q/k projection phases accumulate j-outer across six packed
512-col PSUM regions (phase-scoped pool, bufs=4, full 8 banks) so the
tensor engine consumes each (Wq_j, xT_j) e-tile as it lands; the region
drains alternate vector/scalar so pass boundaries don't serialize on one
engine. Output returns as bf16 (cast to fp32 on host); the last proj
tile's 1/r scaling is split across both engines before the out-DMA.
"""

import os

import numpy as np
import ml_dtypes

import concourse.bass as bass
import concourse.bacc as bacc
import concourse.tile as tile
from concourse import mybir
from concourse.bass_utils import run_bass_kernel_spmd

P = 128
B, T, S, E, H = 8, 1024, 1024, 768, 768
NE, NH, NT, NS = E // P, H // P, T // P, S // P
F32 = mybir.dt.float32
BF16 = mybir.dt.bfloat16
MM_DT = mybir.dt.float32r  # PE fast mode for 4-byte floats
AFT = mybir.ActivationFunctionType

_NC_CACHE = {}
LAST_RESULT = None


def _build_bass():
    nc = bacc.Bacc()
    xT_d = nc.declare_dram_parameter("xT", [E, T], MM_DT, isOutput=False)
    encT_d = nc.declare_dram_parameter("encT", [E, S], MM_DT, isOutput=False)
    wq_d = nc.declare_dram_parameter("wq", [E, H], MM_DT, isOutput=False)
    wk_d = nc.declare_dram_parameter("wk", [E, H], MM_DT, isOutput=False)
    wv_d = nc.declare_dram_parameter("wv", [E, H], MM_DT, isOutput=False)
    wp_d = nc.declare_dram_parameter("wp", [H, E], MM_DT, isOutput=False)
    bqk_d = nc.declare_dram_parameter("bqk", [P, 2 * NH], F32, isOutput=False)
    out_d = nc.declare_dram_parameter("out", [T, E], BF16, isOutput=True)
    rrow_d = nc.dram_tensor("rrow_bounce", [1, T], F32)

    def mm(ps, lhsT, rhs, start, stop):
        nc.tensor.matmul(ps, lhsT, rhs, start=start, stop=stop)

    with tile.TileContext(nc) as tc:
        with (
            tc.tile_pool(name="const", bufs=1) as constp,
            tc.tile_pool(name="big", bufs=1) as bigp,
            tc.tile_pool(name="yout", bufs=3) as youtp,
        ):
            bqk_sb = constp.tile([P, 2 * NH], F32, tag="bqk")
            bq_sb = bqk_sb[:, 0:NH]
            bk_sb = bqk_sb[:, NH:2 * NH]
            ones_stg = constp.tile([P, 2], F32, tag="ones_stg")
            nc.vector.memset(ones_stg[:], 1.0)
            ones_sb = constp.tile([P, 2], MM_DT, tag="ones")
            nc.vector.tensor_copy(ones_sb[:], ones_stg[:])
            rcp_sb = constp.tile([P, NT], F32, tag="rcp")
            rrow_sb = constp.tile([P, T], F32, tag="rrow")

            # long-lived activations (kT/v are stationary-side: bf16)
            qT_sb = bigp.tile([P, NH * T], MM_DT, tag="qT")
            kT_sb = bigp.tile([P, NH * S], MM_DT, tag="kT")
            v_sb = bigp.tile([P, NS * H], MM_DT, tag="v")

            with (
                tc.tile_pool(name="ph1", bufs=1) as ph1,
                tc.tile_pool(name="ps1", bufs=4, space="PSUM") as ps1,
            ):
                wq_sb = ph1.tile([P, NE * H], MM_DT, tag="wq")
                xT_sb = ph1.tile([P, NE * T], MM_DT, tag="xT")
                wk_sb = ph1.tile([P, NE * H], MM_DT, tag="wk")
                encT_sb = ph1.tile([P, NE * S], MM_DT, tag="encT")
                wv_sb = ph1.tile([P, NE * H], MM_DT, tag="wv")
                def view_of(dram, j):
                    return dram[:].rearrange("(j p) t -> j p t", p=P)[j]

                # one trigger per e-tile: DMA triggers issue serially at
                # ~640ns each on the Sync engine, so fewer+bigger wins; the
                # transfers themselves are packet-spread across 16 engines.
                # xT j=0 is split in half so the very first accumulation
                # round (j=0, h0=0) is gated on less data.
                nc.sync.dma_start(wq_sb[:, 0:H], view_of(wq_d, 0))
                xv0 = view_of(xT_d, 0)
                nc.sync.dma_start(xT_sb[:, 0:512], xv0[:, 0:512])
                nc.sync.dma_start(xT_sb[:, 512:T], xv0[:, 512:T])
                for j in range(1, NE):
                    nc.sync.dma_start(wq_sb[:, j * H:(j + 1) * H], view_of(wq_d, j))
                    nc.sync.dma_start(xT_sb[:, j * T:(j + 1) * T], view_of(xT_d, j))
                # bias gathers are descriptor-heavy; keep them off the
                # critical first trigger slots
                nc.sync.dma_start(bqk_sb[:], bqk_d[:])
                for j in range(NE):
                    nc.sync.dma_start(wk_sb[:, j * H:(j + 1) * H], view_of(wk_d, j))
                    nc.sync.dma_start(encT_sb[:, j * S:(j + 1) * S], view_of(encT_d, j))
                for j in range(NE):
                    nc.sync.dma_start(wv_sb[:, j * H:(j + 1) * H], view_of(wv_d, j))

                # PE warm-up: the tensor engine only reaches full clock
                # after ~3us of continuous execution, and the first real
                # matmul can't start until its operands land (~10us). Spin
                # the PE on junk matmuls with no DMA dependency (gpsimd
                # prepares the operand at ~6.5us) so the p-state ramp is
                # paid during the DMA wait instead of during the q phase.
                warm_stg = ph1.tile([P, 512], F32, tag="warm_stg")
                nc.gpsimd.memset(warm_stg[:], 0.0)
                warm_src = ph1.tile([P, 512], MM_DT, tag="warm")
                nc.vector.tensor_copy(warm_src[:], warm_stg[:])
                pw = ps1.tile([P, T], F32, tag="mm", name="warm_ps")
                for _ in range(6):
                    nc.tensor.matmul(pw[0:2, 0:512], ones_sb[:], warm_src[:],
                                     start=True, stop=True)

                # qT / kT: j-outer accumulation so the PE consumes operand
                # e-tiles in DMA-arrival order. Six 512-col accumulation
                # regions live at once, packed two per [P, T] psum tile;
                # bufs=4 lets the next pass start while this one drains, and
                # the drains alternate vector/scalar so neither engine gates.
                def proj_qk(w_sb, src_sb, dst_sb, b_cols, width):
                    for h0 in range(0, width, 512):
                        pst = [ps1.tile([P, T], F32, tag="mm", name=f"qk_acc{h0}_{u}")
                               for u in range(3)]
                        def acc(i):
                            return pst[i // 2][:, (i % 2) * 512:(i % 2) * 512 + 512]
                        for j in range(NE):
                            iorder = range(NH) if j < NE - 1 else range(NH - 1, -1, -1)
                            for i in iorder:
                                mm(acc(i),
                                   w_sb[:, j * H + i * P: j * H + (i + 1) * P],
                                   src_sb[:, j * width + h0: j * width + h0 + 512],
                                   start=(j == 0), stop=(j == NE - 1))
                        for i in range(NH - 1, -1, -1):
                            dst = dst_sb[:, i * width + h0: i * width + h0 + 512]
                            if i % 2 == 0:
                                nc.vector.tensor_scalar_add(
                                    dst, acc(i), b_cols[i])
                            else:
                                nc.scalar.activation(
                                    dst, acc(i), AFT.Identity,
                                    bias=b_cols[i])

                proj_qk(wq_sb, xT_sb, qT_sb,
                        [bqk_sb[:, i:i + 1] for i in range(NH)], T)
                proj_qk(wk_sb, encT_sb, kT_sb,
                        [bqk_sb[:, NH + i:NH + i + 1] for i in range(NH)], S)

                # v[s-tile si] = sum_j encT[e_j, s_si]^T @ Wv[e_j, :]
                for si in range(NS):
                    ps = ps1.tile([P, H], F32, tag="mm")
                    for n0, n1 in ((0, 512), (512, H)):
                        for j in range(NE):
                            mm(ps[:, n0:n1],
                               encT_sb[:, j * S + si * P: j * S + (si + 1) * P],
                               wv_sb[:, j * H + n0: j * H + n1],
                               start=(j == 0), stop=(j == NE - 1))
                    if si == NS - 1:
                        nc.scalar.copy(v_sb[:, si * H:si * H + 384], ps[:, 0:384])
                        nc.vector.tensor_copy(
                            v_sb[:, si * H + 384:(si + 1) * H], ps[:, 384:H])
                    elif si % 2 == 0:
                        nc.scalar.copy(v_sb[:, si * H:(si + 1) * H], ps[:])
                    else:
                        nc.vector.tensor_copy(v_sb[:, si * H:(si + 1) * H], ps[:])

            with (
                tc.tile_pool(name="ph2", bufs=1) as ph2,
                tc.tile_pool(name="psum", bufs=3, space="PSUM") as psp,
                tc.tile_pool(name="psum_r", bufs=1, space="PSUM") as psr,
            ):
                wp_sb = ph2.tile([P, NH * E], MM_DT, tag="wp")
                for j in range(NH):
                    nc.sync.dma_start(
                        wp_sb[:, j * E:(j + 1) * E],
                        wp_d[:].rearrange("(j p) e -> j p e", p=P)[j])
                ex_sb = ph2.tile([P, NS * T], MM_DT, tag="ex")
                ot_sb = ph2.tile([P, NH * T], MM_DT, tag="ot")

                # ST[s-tile si] = sum_i kT[h_i, s_si]^T @ qT[h_i, :]; Ex = exp
                for si in range(NS):
                    ps = psp.tile([P, T], F32, tag="mm")
                    for h0 in range(0, T, 512):
                        for i in range(NH):
                            mm(ps[:, h0:h0 + 512],
                               kT_sb[:, i * S + si * P: i * S + (si + 1) * P],
                               qT_sb[:, i * T + h0: i * T + h0 + 512],
                               start=(i == 0), stop=(i == NH - 1))
                    nc.scalar.activation(
                        ex_sb[:, si * T:(si + 1) * T], ps[:], AFT.Exp)

                # r[t] = ones^T @ Ex accumulated over s-tiles -> row [2, T]
                # (ones is the 2-col stationary operand so each of the 16
                # matmuls streams 512 rows instead of paying a 128-col
                # LDWEIGHTS for 1 row of output)
                pr = psr.tile([2, T], F32, tag="r")
                for h0 in range(0, T, 512):
                    for si in range(NS):
                        nc.tensor.matmul(
                            pr[:, h0:h0 + 512],
                            ones_sb[:],
                            ex_sb[:, si * T + h0: si * T + h0 + 512],
                            start=(si == 0), stop=(si == NS - 1))
                nc.vector.reciprocal(rrow_sb[0:1, :], pr[0:1, :])

                # OT[h-tile i] = sum_si v[s_si, h_i]^T @ Ex[s_si, :]
                for i in range(NH):
                    ps = psp.tile([P, T], F32, tag="mm")
                    for h0 in range(0, T, 512):
                        for si in range(NS):
                            mm(ps[:, h0:h0 + 512],
                               v_sb[:, si * H + i * P: si * H + (i + 1) * P],
                               ex_sb[:, si * T + h0: si * T + h0 + 512],
                               start=(si == 0), stop=(si == NS - 1))
                    if i % 2 == 0:
                        nc.scalar.copy(ot_sb[:, i * T:(i + 1) * T], ps[:])
                    else:
                        nc.vector.tensor_copy(ot_sb[:, i * T:(i + 1) * T], ps[:])

                # scatter the reciprocal row [1, T] into per-partition
                # columns [128, NT] entirely off the PE: bounce the row to
                # DRAM and gather it back partition-strided. Runs ~35us
                # before the proj phase needs rcp, so the latency (~2us,
                # 1024 4-byte read packets) is fully hidden.
                nc.sync.dma_start(rrow_d[:], rrow_sb[0:1, :])
                nc.sync.dma_start(
                    rcp_sb[:, 0:NT],
                    rrow_d[0].rearrange("(ti p) -> p ti", p=P))

                # Y[t-tile ti] = (sum_i OT[h_i, t_ti]^T @ Wp[h_i, :]) * rcp[ti]
                # Alternate the 1/r scaling between vector and scalar; the
                # last tile is split across both engines + two out-DMAs so
                # the tail critical chain after the final matmul is short.
                for ti in range(NT):
                    ps = psp.tile([P, E], F32, tag="mm")
                    for n0, n1 in ((0, 512), (512, E)):
                        for i in range(NH):
                            mm(ps[:, n0:n1],
                               ot_sb[:, i * T + ti * P: i * T + (ti + 1) * P],
                               wp_sb[:, i * E + n0: i * E + n1],
                               start=(i == 0), stop=(i == NH - 1))
                    y_sb = youtp.tile([P, E], BF16, tag="y")
                    if ti == NT - 1:
                        nc.vector.tensor_scalar_mul(
                            y_sb[:, 0:512], ps[:, 0:512], rcp_sb[:, ti:ti + 1])
                        nc.scalar.activation(
                            y_sb[:, 512:E], ps[:, 512:E], AFT.Copy,
                            scale=rcp_sb[:, ti:ti + 1])
                        nc.sync.dma_start(
                            out_d[ti * P:(ti + 1) * P, 0:512], y_sb[:, 0:512])
                        nc.sync.dma_start(
                            out_d[ti * P:(ti + 1) * P, 512:E], y_sb[:, 512:E])
                    else:
                        if ti % 2 == 0:
                            nc.vector.tensor_scalar_mul(
                                y_sb[:], ps[:], rcp_sb[:, ti:ti + 1])
                        else:
                            nc.scalar.activation(
                                y_sb[:], ps[:], AFT.Copy,
                                scale=rcp_sb[:, ti:ti + 1])
                        nc.sync.dma_start(out_d[ti * P:(ti + 1) * P, :], y_sb[:])
    nc.finalize()
    return nc


def get_nc():
    if "nc" not in _NC_CACHE:
        _NC_CACHE["nc"] = _build_bass()
    return _NC_CACHE["nc"]


def kernel(**inputs):
    global LAST_RESULT
    x = np.asarray(inputs["x"], dtype=np.float32)
    enc = np.asarray(inputs["encoder_out"], dtype=np.float32)
    Wq = np.asarray(inputs["Wq"], dtype=np.float32)
    bq = np.asarray(inputs["bq"], dtype=np.float32)
    Wk = np.asarray(inputs["Wk"], dtype=np.float32)
    bk = np.asarray(inputs["bk"], dtype=np.float32)
    Wv = np.asarray(inputs["Wv"], dtype=np.float32)
    bv = np.asarray(inputs["bv"], dtype=np.float32)
    Wp = np.asarray(inputs["Wp"], dtype=np.float32)
    bp = np.asarray(inputs["bp"], dtype=np.float32)

    bf = ml_dtypes.bfloat16
    scale = np.float32(1.0 / np.sqrt(H))
    wq_s = (Wq * scale).astype(np.float32)
    bq_s = (bq * scale).astype(np.float32)
    cvec = (bv @ Wp + bp).astype(np.float32)  # exact rank-1 fold, see header
    bqk = np.ascontiguousarray(np.concatenate(
        [bq_s.reshape(NH, P).T, bk.reshape(NH, P).T], axis=1))
    xT = np.ascontiguousarray(x.transpose(0, 2, 1))
    encT = np.ascontiguousarray(enc.transpose(0, 2, 1))
    
    nc = get_nc()
    in_maps = [
        {"xT": xT[i], "encT": encT[i], "wq": wq_s, "wk": Wk, "wv": Wv,
         "wp": Wp, "bqk": bqk}
        for i in range(B)
    ]
    res = run_bass_kernel_spmd(
        nc, in_maps, list(range(B)),
        trace=bool(os.environ.get("KERNEL_TRACE")),
    )
    LAST_RESULT = res
    out = np.stack([res.results[i]["out"] for i in range(B)]).astype(np.float32)
    if cvec.any():
        out = out + cvec
    return out



# revision 5
# speedup vs baseline: 1.1017x; 1.1017x over previous
"""Single-head cross-attention on 8 NeuronCores, data-parallel over batch.

Math per core (batch element b):
    q = x @ Wq + bq;  k = enc @ Wk + bk;  v = enc @ Wv + bv
    out = softmax(q k^T / sqrt(H)) @ v @ Wp + bp

Layout strategy (no on-chip transposes anywhere):
    host:    xT[E,T], encT[E,S] (pre-transposed), Wq' = Wq/sqrt(H)
    qT[h,t] = Wq'^T-tiles as lhsT, xT as rhs          (+bq' per-partition)
    kT[h,s] = Wk-tiles  as lhsT, encT as rhs          (+bk  per-partition)
    v[s,h]  = encT-tiles as lhsT, Wv as rhs
    ST[s,t] = kT-tiles  as lhsT, qT as rhs            (scores, transposed)
    Ex      = exp(ST)          (no max subtraction; scores are O(1) here,
                                softmax is shift-invariant so result matches)
    r[t]    = ones^T column matmuls over Ex s-tiles   ([t,1] per t-tile)
    OT[h,t] = v-tiles   as lhsT, Ex as rhs            (unnormalized)
    Y[t,e]  = OT-tiles  as lhsT, Wp as rhs, scaled by 1/r[t] on copy-out
    bv/bp are folded into a host-side rank-1 add: softmax rows sum to 1, so
    P@(v + 1 bv^T) @ Wp + bp = P@v@Wp + 1 (bv@Wp + bp)^T exactly.

All matmuls run in float32r (fp32 bits, fast PE mode, 1 cycle/row at
N>=256; measured faster than bf16 on hw: 389ns vs 454ns per 512-col
matmul, and the BIR verifier forbids mixing fp32r with bf16 operands).
The q/k projection phases accumulate j-outer across six packed
512-col PSUM regions (phase-scoped pool, bufs=4, full 8 banks) so the
tensor engine consumes each (Wq_j, xT_j) e-tile as it lands; the region
drains alternate vector/scalar so pass boundaries don't serialize on one
engine. Output returns as bf16 (cast to fp32 on host); the last proj
tile's 1/r scaling is split across both engines before the out-DMA.
"""

import os

import numpy as np
import ml_dtypes

import concourse.bass as bass
import concourse.bacc as bacc
import concourse.tile as tile
from concourse import mybir
from concourse.bass_utils import run_bass_kernel_spmd

P = 128
B, T, S, E, H = 8, 1024, 1024, 768, 768
NE, NH, NT, NS = E // P, H // P, T // P, S // P
F32 = mybir.dt.float32
BF16 = mybir.dt.bfloat16
MM_DT = mybir.dt.bfloat16  # halves DMA traffic; FWL hides LDWEIGHTS
AFT = mybir.ActivationFunctionType

_NC_CACHE = {}
LAST_RESULT = None


def _build_bass():
    nc = bacc.Bacc()
    xT_d = nc.declare_dram_parameter("xT", [E, T], MM_DT, isOutput=False)
    encT_d = nc.declare_dram_parameter("encT", [E, S], MM_DT, isOutput=False)
    wq_d = nc.declare_dram_parameter("wq", [E, H], MM_DT, isOutput=False)
    wk_d = nc.declare_dram_parameter("wk", [E, H], MM_DT, isOutput=False)
    wv_d = nc.declare_dram_parameter("wv", [E, H], MM_DT, isOutput=False)
    wp_d = nc.declare_dram_parameter("wp", [H, E], MM_DT, isOutput=False)
    bqk_d = nc.declare_dram_parameter("bqk", [P, 2 * NH], F32, isOutput=False)
    out_d = nc.declare_dram_parameter("out", [T, E], BF16, isOutput=True)
    rrow_d = nc.dram_tensor("rrow_bounce", [1, T], F32)

    def mm(ps, lhsT, rhs, start, stop):
        nc.tensor.matmul(ps, lhsT, rhs, start=start, stop=stop)

    with tile.TileContext(nc) as tc:
        with (
            tc.tile_pool(name="const", bufs=1) as constp,
            tc.tile_pool(name="big", bufs=1) as bigp,
            tc.tile_pool(name="yout", bufs=3) as youtp,
        ):
            bqk_sb = constp.tile([P, 2 * NH], F32, tag="bqk")
            bq_sb = bqk_sb[:, 0:NH]
            bk_sb = bqk_sb[:, NH:2 * NH]
            ones_sb = constp.tile([P, 2], MM_DT, tag="ones")
            nc.vector.memset(ones_sb[:], 1.0)
            rcp_sb = constp.tile([P, NT], F32, tag="rcp")
            rrow_sb = constp.tile([P, T], F32, tag="rrow")

            # long-lived activations (kT/v are stationary-side: bf16)
            qT_sb = bigp.tile([P, NH * T], MM_DT, tag="qT")
            kT_sb = bigp.tile([P, NH * S], MM_DT, tag="kT")
            v_sb = bigp.tile([P, NS * H], MM_DT, tag="v")

            with (
                tc.tile_pool(name="ph1", bufs=1) as ph1,
                tc.tile_pool(name="ps1", bufs=4, space="PSUM") as ps1,
            ):
                wq_sb = ph1.tile([P, NE * H], MM_DT, tag="wq")
                xT_sb = ph1.tile([P, NE * T], MM_DT, tag="xT")
                wk_sb = ph1.tile([P, NE * H], MM_DT, tag="wk")
                encT_sb = ph1.tile([P, NE * S], MM_DT, tag="encT")
                wv_sb = ph1.tile([P, NE * H], MM_DT, tag="wv")
                def view_of(dram, j):
                    return dram[:].rearrange("(j p) t -> j p t", p=P)[j]

                # one trigger per e-tile: DMA triggers issue serially at
                # ~640ns each on the Sync engine, so fewer+bigger wins; the
                # transfers themselves are packet-spread across 16 engines.
                # xT j=0 is split in half so the very first accumulation
                # round (j=0, h0=0) is gated on less data.
                nc.sync.dma_start(wq_sb[:, 0:H], view_of(wq_d, 0))
                xv0 = view_of(xT_d, 0)
                nc.sync.dma_start(xT_sb[:, 0:512], xv0[:, 0:512])
                nc.sync.dma_start(xT_sb[:, 512:T], xv0[:, 512:T])
                for j in range(1, NE):
                    nc.sync.dma_start(wq_sb[:, j * H:(j + 1) * H], view_of(wq_d, j))
                    nc.sync.dma_start(xT_sb[:, j * T:(j + 1) * T], view_of(xT_d, j))
                # bias gathers are descriptor-heavy; keep them off the
                # critical first trigger slots
                nc.sync.dma_start(bqk_sb[:], bqk_d[:])
                for j in range(NE):
                    nc.sync.dma_start(wk_sb[:, j * H:(j + 1) * H], view_of(wk_d, j))
                    nc.sync.dma_start(encT_sb[:, j * S:(j + 1) * S], view_of(encT_d, j))
                for j in range(NE):
                    nc.sync.dma_start(wv_sb[:, j * H:(j + 1) * H], view_of(wv_d, j))

                # PE warm-up: the tensor engine only reaches full clock
                # after ~3us of continuous execution, and the first real
                # matmul can't start until its operands land (~10us). Spin
                # the PE on junk matmuls with no DMA dependency (gpsimd
                # prepares the operand at ~6.5us) so the p-state ramp is
                # paid during the DMA wait instead of during the q phase.
                warm_src = ph1.tile([P, 512], MM_DT, tag="warm")
                nc.gpsimd.memset(warm_src[:], 0.0)
                pw = ps1.tile([P, T], F32, tag="mm", name="warm_ps")
                for _ in range(6):
                    nc.tensor.matmul(pw[0:2, 0:512], ones_sb[:], warm_src[:],
                                     start=True, stop=True)

                # qT / kT: j-outer accumulation so the PE consumes operand
                # e-tiles in DMA-arrival order. Six 512-col accumulation
                # regions live at once, packed two per [P, T] psum tile;
                # bufs=4 lets the next pass start while this one drains, and
                # the drains alternate vector/scalar so neither engine gates.
                def proj_qk(w_sb, src_sb, dst_sb, b_cols, width):
                    for h0 in range(0, width, 512):
                        pst = [ps1.tile([P, T], F32, tag="mm", name=f"qk_acc{h0}_{u}")
                               for u in range(3)]
                        def acc(i):
                            return pst[i // 2][:, (i % 2) * 512:(i % 2) * 512 + 512]
                        for j in range(NE):
                            iorder = range(NH) if j < NE - 1 else range(NH - 1, -1, -1)
                            for i in iorder:
                                mm(acc(i),
                                   w_sb[:, j * H + i * P: j * H + (i + 1) * P],
                                   src_sb[:, j * width + h0: j * width + h0 + 512],
                                   start=(j == 0), stop=(j == NE - 1))
                        for i in range(NH - 1, -1, -1):
                            dst = dst_sb[:, i * width + h0: i * width + h0 + 512]
                            if i % 2 == 0:
                                nc.vector.tensor_scalar_add(
                                    dst, acc(i), b_cols[i])
                            else:
                                nc.scalar.activation(
                                    dst, acc(i), AFT.Identity,
                                    bias=b_cols[i])

                proj_qk(wq_sb, xT_sb, qT_sb,
                        [bqk_sb[:, i:i + 1] for i in range(NH)], T)
                proj_qk(wk_sb, encT_sb, kT_sb,
                        [bqk_sb[:, NH + i:NH + i + 1] for i in range(NH)], S)

                # v[s-tile si] = sum_j encT[e_j, s_si]^T @ Wv[e_j, :]
                for si in range(NS):
                    ps = ps1.tile([P, H], F32, tag="mm")
                    for n0, n1 in ((0, 512), (512, H)):
                        for j in range(NE):
                            mm(ps[:, n0:n1],
                               encT_sb[:, j * S + si * P: j * S + (si + 1) * P],
                               wv_sb[:, j * H + n0: j * H + n1],
                               start=(j == 0), stop=(j == NE - 1))
                    if si == NS - 1:
                        nc.scalar.copy(v_sb[:, si * H:si * H + 384], ps[:, 0:384])
                        nc.vector.tensor_copy(
                            v_sb[:, si * H + 384:(si + 1) * H], ps[:, 384:H])
                    elif si % 2 == 0:
                        nc.scalar.copy(v_sb[:, si * H:(si + 1) * H], ps[:])
                    else:
                        nc.vector.tensor_copy(v_sb[:, si * H:(si + 1) * H], ps[:])

            with (
                tc.tile_pool(name="ph2", bufs=1) as ph2,
                tc.tile_pool(name="psum", bufs=3, space="PSUM") as psp,
                tc.tile_pool(name="psum_r", bufs=1, space="PSUM") as psr,
            ):
                wp_sb = ph2.tile([P, NH * E], MM_DT, tag="wp")
                for j in range(NH):
                    nc.sync.dma_start(
                        wp_sb[:, j * E:(j + 1) * E],
                        wp_d[:].rearrange("(j p) e -> j p e", p=P)[j])
                ex_sb = ph2.tile([P, NS * T], MM_DT, tag="ex")
                ot_sb = ph2.tile([P, NH * T], MM_DT, tag="ot")

                # ST[s-tile si] = sum_i kT[h_i, s_si]^T @ qT[h_i, :]; Ex = exp
                for si in range(NS):
                    ps = psp.tile([P, T], F32, tag="mm")
                    for h0 in range(0, T, 512):
                        for i in range(NH):
                            mm(ps[:, h0:h0 + 512],
                               kT_sb[:, i * S + si * P: i * S + (si + 1) * P],
                               qT_sb[:, i * T + h0: i * T + h0 + 512],
                               start=(i == 0), stop=(i == NH - 1))
                    nc.scalar.activation(
                        ex_sb[:, si * T:(si + 1) * T], ps[:], AFT.Exp)

                # r[t] = ones^T @ Ex accumulated over s-tiles -> row [2, T]
                # (ones is the 2-col stationary operand so each of the 16
                # matmuls streams 512 rows instead of paying a 128-col
                # LDWEIGHTS for 1 row of output)
                pr = psr.tile([2, T], F32, tag="r")
                for h0 in range(0, T, 512):
                    for si in range(NS):
                        nc.tensor.matmul(
                            pr[:, h0:h0 + 512],
                            ones_sb[:],
                            ex_sb[:, si * T + h0: si * T + h0 + 512],
                            start=(si == 0), stop=(si == NS - 1))
                nc.vector.reciprocal(rrow_sb[0:1, :], pr[0:1, :])

                # OT[h-tile i] = sum_si v[s_si, h_i]^T @ Ex[s_si, :]
                for i in range(NH):
                    ps = psp.tile([P, T], F32, tag="mm")
                    for h0 in range(0, T, 512):
                        for si in range(NS):
                            mm(ps[:, h0:h0 + 512],
                               v_sb[:, si * H + i * P: si * H + (i + 1) * P],
                               ex_sb[:, si * T + h0: si * T + h0 + 512],
                               start=(si == 0), stop=(si == NS - 1))
                    if i % 2 == 0:
                        nc.scalar.copy(ot_sb[:, i * T:(i + 1) * T], ps[:])
                    else:
                        nc.vector.tensor_copy(ot_sb[:, i * T:(i + 1) * T], ps[:])

                # scatter the reciprocal row [1, T] into per-partition
                # columns [128, NT] entirely off the PE: bounce the row to
                # DRAM and gather it back partition-strided. Runs ~35us
                # before the proj phase needs rcp, so the latency (~2us,
                # 1024 4-byte read packets) is fully hidden.
                nc.sync.dma_start(rrow_d[:], rrow_sb[0:1, :])
                nc.sync.dma_start(
                    rcp_sb[:, 0:NT],
                    rrow_d[0].rearrange("(ti p) -> p ti", p=P))

                # Y[t-tile ti] = (sum_i OT[h_i, t_ti]^T @ Wp[h_i, :]) * rcp[ti]
                # Alternate the 1/r scaling between vector and scalar; the
                # last tile is split across both engines + two out-DMAs so
                # the tail critical chain after the final matmul is short.
                for ti in range(NT):
                    ps = psp.tile([P, E], F32, tag="mm")
                    for n0, n1 in ((0, 512), (512, E)):
                        for i in range(NH):
                            mm(ps[:, n0:n1],
                               ot_sb[:, i * T + ti * P: i * T + (ti + 1) * P],
                               wp_sb[:, i * E + n0: i * E + n1],
                               start=(i == 0), stop=(i == NH - 1))
                    y_sb = youtp.tile([P, E], BF16, tag="y")
                    if ti == NT - 1:
                        nc.vector.tensor_scalar_mul(
                            y_sb[:, 0:512], ps[:, 0:512], rcp_sb[:, ti:ti + 1])
                        nc.scalar.activation(
                            y_sb[:, 512:E], ps[:, 512:E], AFT.Copy,
                            scale=rcp_sb[:, ti:ti + 1])
                        nc.sync.dma_start(
                            out_d[ti * P:(ti + 1) * P, 0:512], y_sb[:, 0:512])
                        nc.sync.dma_start(
                            out_d[ti * P:(ti + 1) * P, 512:E], y_sb[:, 512:E])
                    else:
                        if ti % 2 == 0:
                            nc.vector.tensor_scalar_mul(
                                y_sb[:], ps[:], rcp_sb[:, ti:ti + 1])
                        else:
                            nc.scalar.activation(
                                y_sb[:], ps[:], AFT.Copy,
                                scale=rcp_sb[:, ti:ti + 1])
                        nc.sync.dma_start(out_d[ti * P:(ti + 1) * P, :], y_sb[:])
    nc.finalize()
    return nc


def get_nc():
    if "nc" not in _NC_CACHE:
        _NC_CACHE["nc"] = _build_bass()
    return _NC_CACHE["nc"]


def kernel(**inputs):
    global LAST_RESULT
    x = np.asarray(inputs["x"], dtype=np.float32)
    enc = np.asarray(inputs["encoder_out"], dtype=np.float32)
    Wq = np.asarray(inputs["Wq"], dtype=np.float32)
    bq = np.asarray(inputs["bq"], dtype=np.float32)
    Wk = np.asarray(inputs["Wk"], dtype=np.float32)
    bk = np.asarray(inputs["bk"], dtype=np.float32)
    Wv = np.asarray(inputs["Wv"], dtype=np.float32)
    bv = np.asarray(inputs["bv"], dtype=np.float32)
    Wp = np.asarray(inputs["Wp"], dtype=np.float32)
    bp = np.asarray(inputs["bp"], dtype=np.float32)

    bf = ml_dtypes.bfloat16
    scale = np.float32(1.0 / np.sqrt(H))
    wq_s = (Wq * scale).astype(bf)
    bq_s = (bq * scale).astype(np.float32)
    cvec = (bv @ Wp + bp).astype(np.float32)  # exact rank-1 fold, see header
    bqk = np.ascontiguousarray(np.concatenate(
        [bq_s.reshape(NH, P).T, bk.reshape(NH, P).T], axis=1))
    xT = np.ascontiguousarray(x.transpose(0, 2, 1)).astype(bf)
    encT = np.ascontiguousarray(enc.transpose(0, 2, 1)).astype(bf)
    Wk = Wk.astype(bf)
    Wv = Wv.astype(bf)
    Wp = Wp.astype(bf)
    
    nc = get_nc()
    in_maps = [
        {"xT": xT[i], "encT": encT[i], "wq": wq_s, "wk": Wk, "wv": Wv,
         "wp": Wp, "bqk": bqk}
        for i in range(B)
    ]
    res = run_bass_kernel_spmd(
        nc, in_maps, list(range(B)),
        trace=bool(os.environ.get("KERNEL_TRACE")),
    )
    LAST_RESULT = res
    out = np.stack([res.results[i]["out"] for i in range(B)]).astype(np.float32)
    if cvec.any():
        out = out + cvec
    return out

